# revision 47
# baseline (speedup 1.0000x reference)
"""BlockSparseThresLinear Trainium2 kernel.

Problem (hardcoded): x (128,1,4096) f16, weight (4096,11008) f16, bias (11008,) f16.
  BLOCK_M=16, BLOCK_K=64, THRES=0.8: per (16,64) block of x.reshape(128,4096),
  mask = mean(|block|, fp32) > 0.8; y = (x * mask_expanded) @ weight + bias.

Sharding: weight/bias column-sharded across 8 cores (1376 cols each); x
replicated; each core computes its output slice independently; host concats.

Per-core device pipeline (memory-bound: the 11.27MB W slice stream is the
roofline; cost-model total ~42us vs ~35us pure-DMA floor):
  - W streams on the sync/HWDGE queue only; x in 8 chunk tiles (first on
    the scalar/HWDGE queue, rest on gpsimd/SWDGE) so x never delays W.
  - per x chunk: DVE abs+sum over 64-wide blocks -> bsum [128,8] f32;
    PE matmul with block-diag GG^T (host input) sums each 16-row group
    (broadcast to all rows); DVE is_gt 819.2 -> maskrow {0,1} f16.
  - per K-chunk kc: DVE mul x*mask (step-0 broadcast AP), PE transpose
    (identity from host) -> PSUM f16, DVE copy -> xmT (deep pool so all
    transposes complete ahead of the W stream), 3 PE matmuls accumulate
    per-slice PSUM tiles [128,{512,512,352}] += xmT.T @ w_kc.
  - last two K-chunks stream slice-major so each output slice finishes
    (gemm -> DVE psum copy -> DMA out) while later slices still stream.
  - No ACT compute at all: keeps the scalar queue a pure DMA dispatcher
    (no LoadActFuncSet table load at the head).
"""

import numpy as np

M = 128
K = 4096
N_FULL = 11008
N_CORES = 8
NPC = N_FULL // N_CORES  # 1376
KC = K // 128  # 32 chunks
THRES_SUM = 819.2  # 0.8 * 1024 (exact in fp32: matches (sum/1024) > 0.8f)

_STATE = {}


def _build(bias_nonzero: bool, loop_reps: int = 1):
    from contextlib import ExitStack

    import concourse.bacc as bacc
    import concourse.bass as bass
    import concourse.mybir as mybir
    import concourse.tile as tile

    f16 = mybir.dt.float16
    f32 = mybir.dt.float32

    nc = bacc.Bacc(
        "TRN2",
        target_bir_lowering=False,
        debug=False,
        enable_asserts=False,
        num_devices=N_CORES,
    )

    x = nc.dram_tensor("x", [M, K], f16, kind="ExternalInput").ap()
    w = nc.dram_tensor("w", [K, NPC], f16, kind="ExternalInput").ap()
    b = nc.dram_tensor("b", [1, NPC], f16, kind="ExternalInput").ap()
    gg = nc.dram_tensor("gg", [M, M], f32, kind="ExternalInput").ap()
    idin = nc.dram_tensor("idin", [128, 128], f16, kind="ExternalInput").ap()
    y = nc.dram_tensor("y", [M, NPC], f16, kind="ExternalOutput").ap()

    # Output N split into PSUM-bank-sized slices (<=512 fp32 per bank).
    n_slices = [(0, 512), (512, 1024), (1024, NPC)]

    XCH = 8
    xw = K // XCH  # 512 cols = 8 blocks = 4 K-chunks per x chunk
    KC_G = KC // XCH

    with tile.TileContext(nc) as tc, ExitStack() as ctx:
        if loop_reps > 1:
            # benchmark-only: repeat the whole pipeline on-device so
            # differential wall timing can resolve the per-iteration time
            ctx.enter_context(tc.For_i(0, loop_reps, 1))
        singles = ctx.enter_context(tc.tile_pool(name="singles", bufs=1))
        wpool = ctx.enter_context(tc.tile_pool(name="wpool", bufs=20))
        xmpool = ctx.enter_context(tc.tile_pool(name="xmpool", bufs=8))
        xmtpool = ctx.enter_context(tc.tile_pool(name="xmtpool", bufs=KC))
        mrpool = ctx.enter_context(tc.tile_pool(name="mrpool", bufs=4))
        outpool = ctx.enter_context(tc.tile_pool(name="outpool", bufs=1))
        wlpool = ctx.enter_context(tc.tile_pool(name="wlpool", bufs=2))
        ps_t = ctx.enter_context(tc.tile_pool(name="ps_t", bufs=3, space="PSUM"))
        ps_y = ctx.enter_context(tc.tile_pool(name="ps_y", bufs=1, space="PSUM"))
        ps_m = ctx.enter_context(tc.tile_pool(name="ps_m", bufs=2, space="PSUM"))

        # Prologue DMAs split across independent dispatch resources: x0/gg/
        # ident on the scalar/HWDGE queue, bulk x chunks on gpsimd/SWDGE
        # (parallel dispatcher), so the sync/HWDGE queue carries nothing but
        # the W stream.
        xtiles = []
        for c in range(XCH):
            xsb = singles.tile([M, xw], f16, tag=f"xsb{c}")
            eng = nc.scalar if c == 0 else nc.gpsimd
            eng.dma_start(out=xsb[:], in_=x[:, c * xw : (c + 1) * xw])
            xtiles.append(xsb)

        ggs = singles.tile([M, M], f32)
        nc.scalar.dma_start(out=ggs[:], in_=gg[:])
        ident = singles.tile([128, 128], f16)
        nc.scalar.dma_start(out=ident[:], in_=idin[:])

        if bias_nonzero:
            bias_b = singles.tile([M, NPC], f16)
            bcast = bass.AP(tensor=b.tensor, offset=b.offset, ap=[[0, M], b.ap[1]])
            nc.sync.dma_start(out=bias_b[:], in_=bcast)

        ypsums = {}
        for i, (lo, hi) in enumerate(n_slices):
            yps_tile = ps_y.tile([M, hi - lo], f32, tag=f"ypsum{i}")
            ypsums[lo] = yps_tile
        ysb = outpool.tile([M, NPC], f16)

        def emit_out_range(pk, a, bnd):
            # PSUM[pk] sub-range -> f16 SBUF (+bias) on DVE, then DMA out.
            # DVE-only keeps ACT a pure DMA-dispatch queue (no LoadActFuncSet
            # table load blocking the x0 dispatch).
            if bias_nonzero:
                nc.vector.tensor_tensor(
                    out=ysb[:, a:bnd],
                    in0=ypsums[pk][:, a - pk : bnd - pk],
                    in1=bias_b[:, a:bnd],
                    op=mybir.AluOpType.add,
                )
            else:
                nc.vector.tensor_copy(
                    out=ysb[:, a:bnd], in_=ypsums[pk][:, a - pk : bnd - pk]
                )
            # middle slice on the scalar queue so y dispatches overlap
            eng = nc.scalar if a == 512 else nc.sync
            eng.dma_start(out=y[:, a:bnd], in_=ysb[:, a:bnd])

        xmt_tail = {}
        for c in range(XCH):
            xsb = xtiles[c]
            nbl = xw // 64  # 16 blocks
            bsum = mrpool.tile([M, nbl], f32, tag="bsum")
            nc.vector.tensor_reduce(
                out=bsum[:],
                in_=xsb[:].rearrange("p (b q) -> p b q", q=64),
                axis=mybir.AxisListType.X,
                op=mybir.AluOpType.add,
                apply_absolute_value=True,
            )
            gsum = ps_m.tile([M, nbl], f32)
            nc.tensor.matmul(gsum[:], lhsT=ggs[:], rhs=bsum[:], start=True, stop=True)
            maskrow = mrpool.tile([M, nbl], f16, tag="maskrow")
            nc.vector.tensor_scalar(
                out=maskrow[:],
                in0=gsum[:],
                scalar1=float(THRES_SUM),
                scalar2=None,
                op0=mybir.AluOpType.is_gt,
            )

            for j in range(KC_G):
                kc = c * KC_G + j
                tailk = kc >= KC - 2
                if not tailk:
                    wsb = wpool.tile([128, NPC], f16)
                    nc.sync.dma_start(
                        out=wsb[:], in_=w[kc * 128 : (kc + 1) * 128, :]
                    )

                xm = xmpool.tile([128, 128], f16)
                mview = maskrow[:, 2 * j : 2 * j + 2].unsqueeze(2).broadcast_to(
                    [128, 2, 64]
                )
                nc.vector.tensor_tensor(
                    out=xm[:].rearrange("p (b q) -> p b q", q=64),
                    in0=xsb[:, j * 128 : (j + 1) * 128].rearrange(
                        "p (b q) -> p b q", q=64
                    ),
                    in1=mview,
                    op=mybir.AluOpType.mult,
                )

                pst = ps_t.tile([128, 128], f16)
                nc.tensor.transpose(pst[:], xm[:], ident[:])
                xmt = xmtpool.tile([128, 128], f16)
                nc.vector.tensor_copy(out=xmt[:], in_=pst[:])

                if not tailk:
                    for lo, hi in n_slices:
                        nc.tensor.matmul(
                            ypsums[lo][:],
                            lhsT=xmt[:],
                            rhs=wsb[:, lo:hi],
                            start=(kc == 0),
                            stop=False,
                        )
                else:
                    xmt_tail[kc] = xmt
                    if kc == KC - 1:
                        # Final two K-chunks stream slice-major: each slice's
                        # last gemms -> psum copy -> output DMA pipeline
                        # while later slices still stream.
                        tail_pieces = [
                            (0, 0, 512),
                            (512, 512, 1024),
                            (1024, 1024, NPC),
                        ]
                        for pk, a, bnd in tail_pieces:
                            for kk in (KC - 2, KC - 1):
                                wl = wlpool.tile(
                                    [128, bnd - a], f16, tag=f"wl{a}_{kk % 2}"
                                )
                                nc.sync.dma_start(
                                    out=wl[:],
                                    in_=w[kk * 128 : (kk + 1) * 128, a:bnd],
                                )
                                nc.tensor.matmul(
                                    ypsums[pk][:, a - pk : bnd - pk],
                                    lhsT=xmt_tail[kk][:],
                                    rhs=wl[:],
                                    start=False,
                                    stop=(kk == KC - 1),
                                )
                            emit_out_range(pk, a, bnd)

    nc.compile()
    return nc


def _get_nc(bias_nonzero: bool, loop_reps: int = 1):
    key = ("nc", bias_nonzero, loop_reps)
    if key not in _STATE:
        _STATE[key] = _build(bias_nonzero, loop_reps)
    return _STATE[key]


def _make_in_maps(x, weight, bias):
    x2 = np.ascontiguousarray(np.asarray(x, dtype=np.float16).reshape(M, K))
    wf = np.asarray(weight, dtype=np.float16)
    bf = np.asarray(bias, dtype=np.float16)
    gg = np.kron(np.eye(8, dtype=np.float32), np.ones((16, 16), np.float32))
    ident = np.eye(128, dtype=np.float16)
    in_maps = []
    for c in range(N_CORES):
        in_maps.append(
            {
                "x": x2,
                "w": np.ascontiguousarray(wf[:, c * NPC : (c + 1) * NPC]),
                "b": np.ascontiguousarray(bf[c * NPC : (c + 1) * NPC]).reshape(
                    1, NPC
                ),
                "gg": gg,
                "idin": ident,
            }
        )
    return in_maps


def kernel(x, weight, bias, _trace=False):
    from concourse.bass_utils import run_bass_kernel_spmd

    bias_nonzero = bool(np.any(np.asarray(bias)))
    nc = _get_nc(bias_nonzero)
    in_maps = _make_in_maps(x, weight, bias)
    res = run_bass_kernel_spmd(
        nc, in_maps, core_ids=list(range(N_CORES)), trace=_trace
    )
    _STATE["last_results"] = res
    y = np.concatenate([res.results[c]["y"] for c in range(N_CORES)], axis=1)
    return y.reshape(M, 1, N_FULL).astype(np.float16)


# revision 48
# speedup vs baseline: 1.1695x; 1.1695x over previous
"""BlockSparseThresLinear Trainium2 kernel.

Problem (hardcoded): x (128,1,4096) f16, weight (4096,11008) f16, bias (11008,) f16.
  BLOCK_M=16, BLOCK_K=64, THRES=0.8: per (16,64) block of x.reshape(128,4096),
  mask = mean(|block|, fp32) > 0.8; y = (x * mask_expanded) @ weight + bias.

Sharding: weight/bias column-sharded across 8 cores (1376 cols each); x
replicated; each core computes its output slice independently; host concats.

Per-core device pipeline (memory-bound: the 11.27MB W slice stream is the
roofline; cost-model total ~42us vs ~35us pure-DMA floor):
  - W streams on the sync/HWDGE queue only; x in 8 chunk tiles (first on
    the scalar/HWDGE queue, rest on gpsimd/SWDGE) so x never delays W.
  - per x chunk: DVE abs+sum over 64-wide blocks -> bsum [128,8] f32;
    PE matmul with block-diag GG^T (host input) sums each 16-row group
    (broadcast to all rows); DVE is_gt 819.2 -> maskrow {0,1} f16.
  - per K-chunk kc: DVE mul x*mask (step-0 broadcast AP), PE transpose
    (identity from host) -> PSUM f16, DVE copy -> xmT (deep pool so all
    transposes complete ahead of the W stream), 3 PE matmuls accumulate
    per-slice PSUM tiles [128,{512,512,352}] += xmT.T @ w_kc.
  - last two K-chunks stream slice-major so each output slice finishes
    (gemm -> DVE psum copy -> DMA out) while later slices still stream.
  - No ACT compute at all: keeps the scalar queue a pure DMA dispatcher
    (no LoadActFuncSet table load at the head).
"""

import numpy as np

M = 128
K = 4096
N_FULL = 11008
N_CORES = 8
NPC = N_FULL // N_CORES  # 1376
KC = K // 128  # 32 chunks
THRES_SUM = 819.2  # 0.8 * 1024 (exact in fp32: matches (sum/1024) > 0.8f)

_STATE = {}


def _build(bias_nonzero: bool, loop_reps: int = 1, variant: str = ""):
    from contextlib import ExitStack

    import concourse.bacc as bacc
    import concourse.bass as bass
    import concourse.mybir as mybir
    import concourse.tile as tile

    f16 = mybir.dt.float16
    f32 = mybir.dt.float32

    nc = bacc.Bacc(
        "TRN2",
        target_bir_lowering=False,
        debug=False,
        enable_asserts=False,
        num_devices=N_CORES,
    )

    x = nc.dram_tensor("x", [M, K], f16, kind="ExternalInput").ap()
    w = nc.dram_tensor("w", [K, NPC], f16, kind="ExternalInput").ap()
    b = nc.dram_tensor("b", [1, NPC], f16, kind="ExternalInput").ap()
    gg = nc.dram_tensor("gg", [M, M], f32, kind="ExternalInput").ap()
    idin = nc.dram_tensor("idin", [128, 128], f16, kind="ExternalInput").ap()
    y = nc.dram_tensor("y", [M, NPC], f16, kind="ExternalOutput").ap()

    # Output N split into PSUM-bank-sized slices (<=512 fp32 per bank).
    n_slices = [(0, 512), (512, 1024), (1024, NPC)]

    XCH = 8
    xw = K // XCH  # 512 cols = 8 blocks = 4 K-chunks per x chunk
    KC_G = KC // XCH

    with tile.TileContext(nc) as tc, ExitStack() as ctx:
        if loop_reps > 1:
            # benchmark-only: repeat the whole pipeline on-device so
            # differential wall timing can resolve the per-iteration time
            ctx.enter_context(tc.For_i(0, loop_reps, 1))
        singles = ctx.enter_context(tc.tile_pool(name="singles", bufs=1))
        wpool = ctx.enter_context(tc.tile_pool(name="wpool", bufs=20))
        xmpool = ctx.enter_context(tc.tile_pool(name="xmpool", bufs=8))
        xmtpool = ctx.enter_context(tc.tile_pool(name="xmtpool", bufs=KC))
        mrpool = ctx.enter_context(tc.tile_pool(name="mrpool", bufs=4))
        outpool = ctx.enter_context(tc.tile_pool(name="outpool", bufs=1))
        wlpool = ctx.enter_context(tc.tile_pool(name="wlpool", bufs=2))
        ps_t = ctx.enter_context(tc.tile_pool(name="ps_t", bufs=3, space="PSUM"))
        ps_y = ctx.enter_context(tc.tile_pool(name="ps_y", bufs=1, space="PSUM"))
        ps_m = ctx.enter_context(tc.tile_pool(name="ps_m", bufs=2, space="PSUM"))

        # Prologue DMAs split across independent dispatch resources: x0/gg/
        # ident on the scalar/HWDGE queue, bulk x chunks on gpsimd/SWDGE
        # (parallel dispatcher), so the sync/HWDGE queue carries nothing but
        # the W stream.
        xtiles = []
        for c in range(XCH):
            xsb = singles.tile([M, xw], f16, tag=f"xsb{c}")
            eng = nc.scalar if c == 0 else nc.gpsimd
            eng.dma_start(out=xsb[:], in_=x[:, c * xw : (c + 1) * xw])
            xtiles.append(xsb)

        ggs = singles.tile([M, M], f32)
        nc.scalar.dma_start(out=ggs[:], in_=gg[:])
        ident = singles.tile([128, 128], f16)
        nc.scalar.dma_start(out=ident[:], in_=idin[:])

        if bias_nonzero:
            bias_b = singles.tile([M, NPC], f16)
            bcast = bass.AP(tensor=b.tensor, offset=b.offset, ap=[[0, M], b.ap[1]])
            nc.sync.dma_start(out=bias_b[:], in_=bcast)

        ypsums = {}
        for i, (lo, hi) in enumerate(n_slices):
            yps_tile = ps_y.tile([M, hi - lo], f32, tag=f"ypsum{i}")
            ypsums[lo] = yps_tile
        ysb = outpool.tile([M, NPC], f16)

        def emit_out_range(pk, a, bnd):
            # PSUM[pk] sub-range -> f16 SBUF (+bias) on DVE, then DMA out.
            # DVE-only keeps ACT a pure DMA-dispatch queue (no LoadActFuncSet
            # table load blocking the x0 dispatch).
            if bias_nonzero:
                nc.vector.tensor_tensor(
                    out=ysb[:, a:bnd],
                    in0=ypsums[pk][:, a - pk : bnd - pk],
                    in1=bias_b[:, a:bnd],
                    op=mybir.AluOpType.add,
                )
            else:
                nc.vector.tensor_copy(
                    out=ysb[:, a:bnd], in_=ypsums[pk][:, a - pk : bnd - pk]
                )
            # middle slice on the scalar queue so y dispatches overlap
            eng = nc.scalar if a == 512 else nc.sync
            eng.dma_start(out=y[:, a:bnd], in_=ysb[:, a:bnd])

        xmt_tail = {}
        for c in range(XCH):
            xsb = xtiles[c]
            nbl = xw // 64  # 16 blocks
            bsum = mrpool.tile([M, nbl], f32, tag="bsum")
            nc.vector.tensor_reduce(
                out=bsum[:],
                in_=xsb[:].rearrange("p (b q) -> p b q", q=64),
                axis=mybir.AxisListType.X,
                op=mybir.AluOpType.add,
                apply_absolute_value=True,
            )
            gsum = ps_m.tile([M, nbl], f32)
            nc.tensor.matmul(gsum[:], lhsT=ggs[:], rhs=bsum[:], start=True, stop=True)
            maskrow = mrpool.tile([M, nbl], f16, tag="maskrow")
            nc.vector.tensor_scalar(
                out=maskrow[:],
                in0=gsum[:],
                scalar1=float(THRES_SUM),
                scalar2=None,
                op0=mybir.AluOpType.is_gt,
            )

            wsb2 = None
            for j in range(KC_G):
                kc = c * KC_G + j
                tailk = kc >= KC - 2
                if not tailk and variant == "wpair":
                    # one DMA per K-chunk PAIR (704KB) halves W DMA count
                    if j % 2 == 0:
                        wsb2 = wpool.tile([128, 2, NPC], f16, tag="wsb2")
                        nc.sync.dma_start(
                            out=wsb2[:],
                            in_=w[kc * 128 : (kc + 2) * 128, :].rearrange(
                                "(a p) n -> p a n", p=128
                            ),
                        )
                    wsb = wsb2[:, j % 2, :]
                elif not tailk:
                    wsb_t = wpool.tile([128, NPC], f16, tag="wsb")
                    nc.sync.dma_start(
                        out=wsb_t[:], in_=w[kc * 128 : (kc + 1) * 128, :]
                    )
                    wsb = wsb_t[:]

                xm = xmpool.tile([128, 128], f16)
                mview = maskrow[:, 2 * j : 2 * j + 2].unsqueeze(2).broadcast_to(
                    [128, 2, 64]
                )
                nc.vector.tensor_tensor(
                    out=xm[:].rearrange("p (b q) -> p b q", q=64),
                    in0=xsb[:, j * 128 : (j + 1) * 128].rearrange(
                        "p (b q) -> p b q", q=64
                    ),
                    in1=mview,
                    op=mybir.AluOpType.mult,
                )

                pst = ps_t.tile([128, 128], f16)
                nc.tensor.transpose(pst[:], xm[:], ident[:])
                xmt = xmtpool.tile([128, 128], f16)
                nc.vector.tensor_copy(out=xmt[:], in_=pst[:])

                if not tailk:
                    for lo, hi in n_slices:
                        nc.tensor.matmul(
                            ypsums[lo][:],
                            lhsT=xmt[:],
                            rhs=wsb[:, lo:hi],
                            start=(kc == 0),
                            stop=False,
                        )
                else:
                    xmt_tail[kc] = xmt
                    if kc == KC - 1:
                        # Final two K-chunks stream slice-major: each slice's
                        # last gemms -> psum copy -> output DMA pipeline
                        # while later slices still stream.
                        tail_pieces = [
                            (0, 0, 512),
                            (512, 512, 1024),
                            (1024, 1024, NPC),
                        ]
                        for pk, a, bnd in tail_pieces:
                            for kk in (KC - 2, KC - 1):
                                wl = wlpool.tile(
                                    [128, bnd - a], f16, tag=f"wl{a}_{kk % 2}"
                                )
                                nc.sync.dma_start(
                                    out=wl[:],
                                    in_=w[kk * 128 : (kk + 1) * 128, a:bnd],
                                )
                                nc.tensor.matmul(
                                    ypsums[pk][:, a - pk : bnd - pk],
                                    lhsT=xmt_tail[kk][:],
                                    rhs=wl[:],
                                    start=False,
                                    stop=(kk == KC - 1),
                                )
                            emit_out_range(pk, a, bnd)

    nc.compile()
    return nc


def _get_nc(bias_nonzero: bool, loop_reps: int = 1, variant: str = ""):
    key = ("nc", bias_nonzero, loop_reps, variant)
    if key not in _STATE:
        _STATE[key] = _build(bias_nonzero, loop_reps, variant)
    return _STATE[key]


def _make_in_maps(x, weight, bias):
    x2 = np.ascontiguousarray(np.asarray(x, dtype=np.float16).reshape(M, K))
    wf = np.asarray(weight, dtype=np.float16)
    bf = np.asarray(bias, dtype=np.float16)
    gg = np.kron(np.eye(8, dtype=np.float32), np.ones((16, 16), np.float32))
    ident = np.eye(128, dtype=np.float16)
    in_maps = []
    for c in range(N_CORES):
        in_maps.append(
            {
                "x": x2,
                "w": np.ascontiguousarray(wf[:, c * NPC : (c + 1) * NPC]),
                "b": np.ascontiguousarray(bf[c * NPC : (c + 1) * NPC]).reshape(
                    1, NPC
                ),
                "gg": gg,
                "idin": ident,
            }
        )
    return in_maps


def kernel(x, weight, bias, _trace=False):
    from concourse.bass_utils import run_bass_kernel_spmd

    bias_nonzero = bool(np.any(np.asarray(bias)))
    nc = _get_nc(bias_nonzero)
    in_maps = _make_in_maps(x, weight, bias)
    res = run_bass_kernel_spmd(
        nc, in_maps, core_ids=list(range(N_CORES)), trace=_trace
    )
    _STATE["last_results"] = res
    y = np.concatenate([res.results[c]["y"] for c in range(N_CORES)], axis=1)
    return y.reshape(M, 1, N_FULL).astype(np.float16)


# revision 51
# speedup vs baseline: 1.1929x; 1.0200x over previous
"""BlockSparseThresLinear Trainium2 kernel.

Problem (hardcoded): x (128,1,4096) f16, weight (4096,11008) f16, bias (11008,) f16.
  BLOCK_M=16, BLOCK_K=64, THRES=0.8: per (16,64) block of x.reshape(128,4096),
  mask = mean(|block|, fp32) > 0.8; y = (x * mask_expanded) @ weight + bias.

Sharding: weight/bias column-sharded across 8 cores (1376 cols each); x
replicated; each core computes its output slice independently; host concats.

Per-core device pipeline (memory-bound: the 11.27MB W slice stream is the
roofline; cost-model total ~42us vs ~35us pure-DMA floor):
  - W streams on the sync/HWDGE queue only; x in 8 chunk tiles (first on
    the scalar/HWDGE queue, rest on gpsimd/SWDGE) so x never delays W.
  - per x chunk: DVE abs+sum over 64-wide blocks -> bsum [128,8] f32;
    PE matmul with block-diag GG^T (host input) sums each 16-row group
    (broadcast to all rows); DVE is_gt 819.2 -> maskrow {0,1} f16.
  - per K-chunk kc: DVE mul x*mask (step-0 broadcast AP), PE transpose
    (identity from host) -> PSUM f16, DVE copy -> xmT (deep pool so all
    transposes complete ahead of the W stream), 3 PE matmuls accumulate
    per-slice PSUM tiles [128,{512,512,352}] += xmT.T @ w_kc.
  - last two K-chunks stream slice-major so each output slice finishes
    (gemm -> DVE psum copy -> DMA out) while later slices still stream.
  - No ACT compute at all: keeps the scalar queue a pure DMA dispatcher
    (no LoadActFuncSet table load at the head).
"""

import numpy as np

M = 128
K = 4096
N_FULL = 11008
N_CORES = 8
NPC = N_FULL // N_CORES  # 1376
KC = K // 128  # 32 chunks
THRES_SUM = 819.2  # 0.8 * 1024 (exact in fp32: matches (sum/1024) > 0.8f)

_STATE = {}


def _build(bias_nonzero: bool, loop_reps: int = 1, variant: str = ""):
    from contextlib import ExitStack

    import concourse.bacc as bacc
    import concourse.bass as bass
    import concourse.mybir as mybir
    import concourse.tile as tile

    f16 = mybir.dt.float16
    f32 = mybir.dt.float32

    nc = bacc.Bacc(
        "TRN2",
        target_bir_lowering=False,
        debug=False,
        enable_asserts=False,
        num_devices=N_CORES,
    )

    if variant == "xstrided":
        x = nc.dram_tensor("x", [M, K], f16, kind="ExternalInput").ap()
    else:
        # chunk-major x layout (host repacks): each x chunk DMA reads a
        # contiguous 128KB region -- HW-measured ~2us faster than strided
        x = nc.dram_tensor("x", [K // 512, M, 512], f16, kind="ExternalInput").ap()
    w = nc.dram_tensor("w", [K, NPC], f16, kind="ExternalInput").ap()
    b = nc.dram_tensor("b", [1, NPC], f16, kind="ExternalInput").ap()
    gg = nc.dram_tensor("gg", [M, M], f32, kind="ExternalInput").ap()
    idin = nc.dram_tensor("idin", [128, 128], f16, kind="ExternalInput").ap()
    y = nc.dram_tensor("y", [M, NPC], f16, kind="ExternalOutput").ap()

    # Output N split into PSUM-bank-sized slices (<=512 fp32 per bank).
    n_slices = [(0, 512), (512, 1024), (1024, NPC)]

    XCH = 8
    xw = K // XCH  # 512 cols = 8 blocks = 4 K-chunks per x chunk
    KC_G = KC // XCH

    with tile.TileContext(nc) as tc, ExitStack() as ctx:
        if loop_reps > 1:
            # benchmark-only: repeat the whole pipeline on-device so
            # differential wall timing can resolve the per-iteration time
            ctx.enter_context(tc.For_i(0, loop_reps, 1))
        singles = ctx.enter_context(tc.tile_pool(name="singles", bufs=1))
        wpool = ctx.enter_context(tc.tile_pool(name="wpool", bufs=20))
        xmpool = ctx.enter_context(tc.tile_pool(name="xmpool", bufs=8))
        xmtpool = ctx.enter_context(tc.tile_pool(name="xmtpool", bufs=KC))
        mrpool = ctx.enter_context(tc.tile_pool(name="mrpool", bufs=4))
        outpool = ctx.enter_context(tc.tile_pool(name="outpool", bufs=1))
        wlpool = ctx.enter_context(tc.tile_pool(name="wlpool", bufs=2))
        ps_t = ctx.enter_context(tc.tile_pool(name="ps_t", bufs=3, space="PSUM"))
        ps_y = ctx.enter_context(tc.tile_pool(name="ps_y", bufs=1, space="PSUM"))
        ps_m = ctx.enter_context(tc.tile_pool(name="ps_m", bufs=2, space="PSUM"))

        # Prologue DMAs split across independent dispatch resources: x0/gg/
        # ident on the scalar/HWDGE queue, bulk x chunks on gpsimd/SWDGE
        # (parallel dispatcher), so the sync/HWDGE queue carries nothing but
        # the W stream.
        xtiles = []
        for c in range(XCH):
            xsb = singles.tile([M, xw], f16, tag=f"xsb{c}")
            eng = nc.scalar if c == 0 else nc.gpsimd
            xin = x[:, c * xw : (c + 1) * xw] if variant == "xstrided" else x[c]
            eng.dma_start(out=xsb[:], in_=xin)
            xtiles.append(xsb)

        ggs = singles.tile([M, M], f32)
        nc.scalar.dma_start(out=ggs[:], in_=gg[:])
        ident = singles.tile([128, 128], f16)
        nc.scalar.dma_start(out=ident[:], in_=idin[:])

        if bias_nonzero:
            bias_b = singles.tile([M, NPC], f16)
            bcast = bass.AP(tensor=b.tensor, offset=b.offset, ap=[[0, M], b.ap[1]])
            nc.sync.dma_start(out=bias_b[:], in_=bcast)

        ypsums = {}
        for i, (lo, hi) in enumerate(n_slices):
            yps_tile = ps_y.tile([M, hi - lo], f32, tag=f"ypsum{i}")
            ypsums[lo] = yps_tile
        ysb = outpool.tile([M, NPC], f16)

        def emit_out_range(pk, a, bnd):
            # PSUM[pk] sub-range -> f16 SBUF (+bias) on DVE, then DMA out.
            # DVE-only keeps ACT a pure DMA-dispatch queue (no LoadActFuncSet
            # table load blocking the x0 dispatch).
            if bias_nonzero:
                nc.vector.tensor_tensor(
                    out=ysb[:, a:bnd],
                    in0=ypsums[pk][:, a - pk : bnd - pk],
                    in1=bias_b[:, a:bnd],
                    op=mybir.AluOpType.add,
                )
            else:
                nc.vector.tensor_copy(
                    out=ysb[:, a:bnd], in_=ypsums[pk][:, a - pk : bnd - pk]
                )
            # middle slice on the scalar queue so y dispatches overlap
            eng = nc.scalar if a == 512 else nc.sync
            eng.dma_start(out=y[:, a:bnd], in_=ysb[:, a:bnd])

        xmt_tail = {}
        for c in range(XCH):
            xsb = xtiles[c]
            nbl = xw // 64  # 16 blocks
            bsum = mrpool.tile([M, nbl], f32, tag="bsum")
            nc.vector.tensor_reduce(
                out=bsum[:],
                in_=xsb[:].rearrange("p (b q) -> p b q", q=64),
                axis=mybir.AxisListType.X,
                op=mybir.AluOpType.add,
                apply_absolute_value=True,
            )
            gsum = ps_m.tile([M, nbl], f32)
            nc.tensor.matmul(gsum[:], lhsT=ggs[:], rhs=bsum[:], start=True, stop=True)
            maskrow = mrpool.tile([M, nbl], f16, tag="maskrow")
            nc.vector.tensor_scalar(
                out=maskrow[:],
                in0=gsum[:],
                scalar1=float(THRES_SUM),
                scalar2=None,
                op0=mybir.AluOpType.is_gt,
            )

            wsb2 = None
            for j in range(KC_G):
                kc = c * KC_G + j
                tailk = kc >= KC - 2
                if not tailk and variant == "wpair":
                    # one DMA per K-chunk PAIR (704KB) halves W DMA count
                    if j % 2 == 0:
                        wsb2 = wpool.tile([128, 2, NPC], f16, tag="wsb2")
                        nc.sync.dma_start(
                            out=wsb2[:],
                            in_=w[kc * 128 : (kc + 2) * 128, :].rearrange(
                                "(a p) n -> p a n", p=128
                            ),
                        )
                    wsb = wsb2[:, j % 2, :]
                elif not tailk:
                    wsb_t = wpool.tile([128, NPC], f16, tag="wsb")
                    weng = nc.scalar if (variant == "w2q" and kc % 2) else nc.sync
                    weng.dma_start(
                        out=wsb_t[:], in_=w[kc * 128 : (kc + 1) * 128, :]
                    )
                    wsb = wsb_t[:]

                xm = xmpool.tile([128, 128], f16)
                mview = maskrow[:, 2 * j : 2 * j + 2].unsqueeze(2).broadcast_to(
                    [128, 2, 64]
                )
                nc.vector.tensor_tensor(
                    out=xm[:].rearrange("p (b q) -> p b q", q=64),
                    in0=xsb[:, j * 128 : (j + 1) * 128].rearrange(
                        "p (b q) -> p b q", q=64
                    ),
                    in1=mview,
                    op=mybir.AluOpType.mult,
                )

                pst = ps_t.tile([128, 128], f16)
                nc.tensor.transpose(pst[:], xm[:], ident[:])
                xmt = xmtpool.tile([128, 128], f16)
                nc.vector.tensor_copy(out=xmt[:], in_=pst[:])

                if not tailk:
                    for lo, hi in n_slices:
                        nc.tensor.matmul(
                            ypsums[lo][:],
                            lhsT=xmt[:],
                            rhs=wsb[:, lo:hi],
                            start=(kc == 0),
                            stop=False,
                        )
                else:
                    xmt_tail[kc] = xmt
                    if kc == KC - 1:
                        # Final two K-chunks stream slice-major: each slice's
                        # last gemms -> psum copy -> output DMA pipeline
                        # while later slices still stream.
                        tail_pieces = [
                            (0, 0, 512),
                            (512, 512, 1024),
                            (1024, 1024, NPC),
                        ]
                        for pk, a, bnd in tail_pieces:
                            for kk in (KC - 2, KC - 1):
                                wl = wlpool.tile(
                                    [128, bnd - a], f16, tag=f"wl{a}_{kk % 2}"
                                )
                                wleng = (
                                    nc.scalar
                                    if (variant == "w2q" and kk % 2)
                                    else nc.sync
                                )
                                wleng.dma_start(
                                    out=wl[:],
                                    in_=w[kk * 128 : (kk + 1) * 128, a:bnd],
                                )
                                nc.tensor.matmul(
                                    ypsums[pk][:, a - pk : bnd - pk],
                                    lhsT=xmt_tail[kk][:],
                                    rhs=wl[:],
                                    start=False,
                                    stop=(kk == KC - 1),
                                )
                            emit_out_range(pk, a, bnd)

    nc.compile()
    return nc


def _get_nc(bias_nonzero: bool, loop_reps: int = 1, variant: str = ""):
    key = ("nc", bias_nonzero, loop_reps, variant)
    if key not in _STATE:
        _STATE[key] = _build(bias_nonzero, loop_reps, variant)
    return _STATE[key]


def _make_in_maps(x, weight, bias):
    x2 = np.ascontiguousarray(
        np.asarray(x, dtype=np.float16)
        .reshape(M, K // 512, 512)
        .transpose(1, 0, 2)
    )
    wf = np.asarray(weight, dtype=np.float16)
    bf = np.asarray(bias, dtype=np.float16)
    gg = np.kron(np.eye(8, dtype=np.float32), np.ones((16, 16), np.float32))
    ident = np.eye(128, dtype=np.float16)
    in_maps = []
    for c in range(N_CORES):
        in_maps.append(
            {
                "x": x2,
                "w": np.ascontiguousarray(wf[:, c * NPC : (c + 1) * NPC]),
                "b": np.ascontiguousarray(bf[c * NPC : (c + 1) * NPC]).reshape(
                    1, NPC
                ),
                "gg": gg,
                "idin": ident,
            }
        )
    return in_maps


def kernel(x, weight, bias, _trace=False):
    from concourse.bass_utils import run_bass_kernel_spmd

    bias_nonzero = bool(np.any(np.asarray(bias)))
    nc = _get_nc(bias_nonzero)
    in_maps = _make_in_maps(x, weight, bias)
    res = run_bass_kernel_spmd(
        nc, in_maps, core_ids=list(range(N_CORES)), trace=_trace
    )
    _STATE["last_results"] = res
    y = np.concatenate([res.results[c]["y"] for c in range(N_CORES)], axis=1)
    return y.reshape(M, 1, N_FULL).astype(np.float16)
